# revision 14
# baseline (speedup 1.0000x reference)
"""Trainium2 Bass kernel for nn_DCRKT (knowledge-tracing step).

Strategy: shard the student/batch axis B=128 across 8 NeuronCores (16
students per core); all weights replicated.  Live compute per student:

  qt -> (folded affine A,c) -> LayerNorm -> query proj -> cosine-normalize
     -> sim vs normalized memory_key -> top-10 masked softmax
     -> mastery = attn @ memory_value -> pred = sigmoid(<pq, mastery>)
  adj = thresholded cosine-sim gram of memory_value rows (+ eye)
  updated = gamma * mv + (1-gamma) * ru,  gamma = sigmoid(forget gate)

The response-encoder branch (o_idx/u_idx/score, r_emb, correct/wrong/
unchosen MLPs, resp/state attention) is dead code in the reference (h_t
is never used), so it is skipped entirely.

Host side only does: index gather of q_emb rows, affine weight folding
(O(d^3), batch-independent), memory_key normalization (weights), input
transpose/slicing per core, and output concatenation.  All O(B*C*D)
math runs on device.
"""

import os

import numpy as np

import concourse.bacc as bacc
import concourse.bass as bass
import concourse.mybir as mybir
import concourse.tile as tile
from concourse.bass_utils import run_bass_kernel_spmd

F32 = mybir.dt.float32
AX = mybir.AxisListType
ALU = mybir.AluOpType
ACTF = mybir.ActivationFunctionType

B, NUM_C, NUM_Q, DQ, DG = 128, 1024, 10000, 128, 64
TOPK, EDGE_TH, DECAY, LN_EPS = 10, 0.05, 0.5, 1e-5
NCORES = 8
BC = B // NCORES            # students per core
NCH = NUM_C // 128          # 8 chunks of 128 concepts
NEG_BIG = -3.0e38

# gram matmul input dtype: float32 (exact, 4 cyc/row) or float32r (1 cyc/row)
GRAM_DT = F32

_KV = os.environ.get("KV", "full")  # debug bisection switch
LAST_EXEC_NS = None
LAST_RESULTS = None

_CACHE: dict = {}


def _register_const(nc, value, dtype=F32):
    key = (dtype, value)
    if key in nc.const_aps.aps:
        return
    t = nc.alloc_sbuf_tensor(f"const-{dtype.name}-{value}", [128, 1], dtype)
    nc.gpsimd.memset(t.ap(), value)
    nc.const_aps.aps[key] = t.ap()


def _build(b0: float, tf_coef: float):
    """Build + compile the per-core Bass program. b0/tf_coef are baked scalars."""
    nc = bacc.Bacc()
    for v in (LN_EPS, 1e-24, 1e-12, b0):
        _register_const(nc, float(v))
    nc.all_engine_barrier()

    # ---- per-core DRAM I/O ----
    qtT_d = nc.dram_tensor("qtT", [DQ, BC], F32, kind="ExternalInput")
    AT_d = nc.dram_tensor("AT", [DQ, DQ], F32, kind="ExternalInput")
    c_d = nc.dram_tensor("c_row", [1, DQ], F32, kind="ExternalInput")
    WqT_d = nc.dram_tensor("WqT", [DQ, DG], F32, kind="ExternalInput")
    bq_d = nc.dram_tensor("bq_row", [1, DG], F32, kind="ExternalInput")
    mknT_d = nc.dram_tensor("mknT", [DG, NUM_C], F32, kind="ExternalInput")
    wrep_d = nc.dram_tensor("wrep", [128, DG], F32, kind="ExternalInput")
    eye_d = nc.dram_tensor("eye", [128, 128], F32, kind="ExternalInput")
    ones_d = nc.dram_tensor("ones_row", [1, BC], F32, kind="ExternalInput")
    dt_d = nc.dram_tensor("delta_t", [BC, NUM_C], F32, kind="ExternalInput")
    mv_d = nc.dram_tensor("mv", [BC, NUM_C, DG], F32, kind="ExternalInput")
    ru_d = nc.dram_tensor("ru", [BC, NUM_C, DG], F32, kind="ExternalInput")

    pred_d = nc.dram_tensor("pred", [BC, 1], F32, kind="ExternalOutput")
    adj_d = nc.dram_tensor("adj", [BC, NUM_C, NUM_C], F32, kind="ExternalOutput")
    upd_d = nc.dram_tensor("upd", [BC, NUM_C, DG], F32, kind="ExternalOutput")

    with tile.TileContext(nc) as tc:
        with (
            tc.tile_pool(name="const", bufs=1) as cp,
            tc.tile_pool(name="npool", bufs=2) as npool,
            tc.tile_pool(name="chk", bufs=3) as chk,
            tc.tile_pool(name="tmp", bufs=3) as tmp,
            tc.tile_pool(name="adjp", bufs=4) as adjp,
        ):
            # ---- load constants ----
            qtT = cp.tile([DQ, BC], F32)
            nc.gpsimd.dma_start(qtT[:], qtT_d[:])
            AT = cp.tile([DQ, DQ], F32)
            nc.gpsimd.dma_start(AT[:], AT_d[:])
            c_row = cp.tile([1, DQ], F32)
            nc.gpsimd.dma_start(c_row[:], c_d[:])
            WqT = cp.tile([DQ, DG], F32)
            nc.gpsimd.dma_start(WqT[:], WqT_d[:])
            bq_row = cp.tile([1, DG], F32)
            nc.gpsimd.dma_start(bq_row[:], bq_d[:])
            mknT = cp.tile([DG, NUM_C], F32)
            nc.gpsimd.dma_start(mknT[:], mknT_d[:])
            wrep = cp.tile([128, DG], F32)
            nc.gpsimd.dma_start(wrep[:], wrep_d[:])
            eye = cp.tile([128, 128], F32)
            nc.gpsimd.dma_start(eye[:], eye_d[:])
            ones_row = cp.tile([1, BC], F32)
            nc.gpsimd.dma_start(ones_row[:], ones_d[:])
            dt_sb = cp.tile([BC, NUM_C], F32)
            nc.gpsimd.dma_start(dt_sb[:], dt_d[:])

            eye16 = eye[:BC, :BC]

            # ================= stage A: qt_hat / pq  =================
            with tc.tile_pool(name="psA", bufs=3, space="PSUM") as psA:
                x_ps = psA.tile([BC, DQ], F32, tag="ps")
                nc.tensor.matmul(x_ps[:], qtT[:], AT[:], start=True, stop=False)
                nc.tensor.matmul(x_ps[:], ones_row[:], c_row[:], start=False, stop=True)
                x_sb = cp.tile([BC, DQ], F32)
                nc.scalar.copy(x_sb[:], x_ps[:])

                mu = cp.tile([BC, 1], F32)
                nc.vector.tensor_reduce(mu[:], x_sb[:], axis=AX.X, op=ALU.add)
                nc.scalar.mul(mu[:], mu[:], 1.0 / DQ)
                xc = cp.tile([BC, DQ], F32)
                nc.vector.tensor_scalar_sub(xc[:], x_sb[:], mu[:])
                sqj = cp.tile([BC, DQ], F32)
                ss = cp.tile([BC, 1], F32)
                nc.scalar.activation(sqj[:], xc[:], ACTF.Square, accum_out=ss[:])
                sd = cp.tile([BC, 1], F32)
                nc.scalar.activation(sd[:], ss[:], ACTF.Sqrt, bias=LN_EPS, scale=1.0 / DQ)
                rstd = cp.tile([BC, 1], F32)
                nc.vector.reciprocal(rstd[:], sd[:])
                z = cp.tile([BC, DQ], F32)
                nc.vector.tensor_scalar_mul(z[:], xc[:], rstd[:])

                zT_ps = psA.tile([DQ, BC], F32, tag="ps")
                nc.tensor.transpose(zT_ps[:], z[:], eye16)
                zT = cp.tile([DQ, BC], F32)
                nc.scalar.copy(zT[:], zT_ps[:])

                pq_ps = psA.tile([BC, DG], F32, tag="ps")
                nc.tensor.matmul(pq_ps[:], zT[:], WqT[:], start=True, stop=False)
                nc.tensor.matmul(pq_ps[:], ones_row[:], bq_row[:], start=False, stop=True)
                pq_sb = cp.tile([BC, DG], F32)
                nc.scalar.copy(pq_sb[:], pq_ps[:])

                sq2 = cp.tile([BC, DG], F32)
                ss2 = cp.tile([BC, 1], F32)
                nc.scalar.activation(sq2[:], pq_sb[:], ACTF.Square, accum_out=ss2[:])
                sn2 = cp.tile([BC, 1], F32)
                nc.scalar.activation(sn2[:], ss2[:], ACTF.Sqrt, bias=1e-24)
                rn2 = cp.tile([BC, 1], F32)
                nc.vector.reciprocal(rn2[:], sn2[:])
                pqn = cp.tile([BC, DG], F32)
                nc.vector.tensor_scalar_mul(pqn[:], pq_sb[:], rn2[:])

                pqnT_ps = psA.tile([DG, BC], F32, tag="ps")
                nc.tensor.transpose(pqnT_ps[:], pqn[:], eye16)
                pqnT = cp.tile([DG, BC], F32)
                nc.scalar.copy(pqnT[:], pqnT_ps[:])

                # sim = pqn @ mkn.T : [BC, NUM_C]
                sim_ps = psA.tile([BC, NUM_C], F32, tag="ps")
                nc.tensor.matmul(sim_ps[:, 0:512], pqnT[:], mknT[:, 0:512], start=True, stop=True)
                nc.tensor.matmul(sim_ps[:, 512:1024], pqnT[:], mknT[:, 512:1024], start=True, stop=True)
                sim_sb = cp.tile([BC, NUM_C], F32)
                nc.scalar.copy(sim_sb[:], sim_ps[:])

                # ============== stage C: top-10 masked softmax ==============
                cur = cp.tile([BC, NUM_C], F32)
                nc.vector.tensor_copy(cur[:], sim_sb[:])
                neg = cp.tile([BC, NUM_C], F32)
                nc.gpsimd.memset(neg[:], NEG_BIG)
                m0 = cp.tile([BC, 1], F32)
                thr = cp.tile([BC, 1], F32)
                for i in range(TOPK):
                    tgt = m0 if i == 0 else thr
                    nc.vector.tensor_reduce(tgt[:], cur[:], axis=AX.X, op=ALU.max)
                    if i < TOPK - 1:
                        msk = cp.tile([BC, NUM_C], mybir.dt.uint8, tag="topk_msk")
                        nc.vector.tensor_single_scalar(msk[:], cur[:], tgt[:], ALU.is_ge)
                        nc.vector.copy_predicated(cur[:], msk[:], neg[:])

                negm0 = cp.tile([BC, 1], F32)
                nc.scalar.mul(negm0[:], m0[:], -1.0)
                ex = cp.tile([BC, NUM_C], F32)
                nc.scalar.activation(ex[:], sim_sb[:], ACTF.Exp, bias=negm0[:])
                mskf = cp.tile([BC, NUM_C], F32)
                nc.vector.tensor_single_scalar(mskf[:], sim_sb[:], thr[:], ALU.is_ge)
                w_sb = cp.tile([BC, NUM_C], F32)
                nc.vector.tensor_mul(w_sb[:], ex[:], mskf[:])
                ssum = cp.tile([BC, 1], F32)
                nc.vector.tensor_reduce(ssum[:], w_sb[:], axis=AX.X, op=ALU.add)
                rs = cp.tile([BC, 1], F32)
                nc.vector.reciprocal(rs[:], ssum[:])

                # attn columns: wT[:, k*BC + b] = w[b, k*128 : (k+1)*128]
                wT = cp.tile([128, NCH * BC], F32)
                for k in range(NCH):
                    wT_ps = psA.tile([128, BC], F32, tag="ps")
                    nc.tensor.transpose(wT_ps[:], w_sb[:, k * 128:(k + 1) * 128], eye16)
                    nc.scalar.copy(wT[:, k * BC:(k + 1) * BC], wT_ps[:])

                # tfT[:, k*BC + b] = log1p(delta_t)[b, k*128 : (k+1)*128]
                tf_sb = cp.tile([BC, NUM_C], F32)
                nc.scalar.activation(tf_sb[:], dt_sb[:], ACTF.Ln, bias=1.0)
                tfT = cp.tile([128, NCH * BC], F32)
                for k in range(NCH):
                    tfT_ps = psA.tile([128, BC], F32, tag="ps")
                    nc.tensor.transpose(tfT_ps[:], tf_sb[:, k * 128:(k + 1) * 128], eye16)
                    nc.scalar.copy(tfT[:, k * BC:(k + 1) * BC], tfT_ps[:])

            # ============== main per-student loop ==============
            mast_sb = cp.tile([BC, DG], F32)
            with (
                tc.tile_pool(name="gram", bufs=4, space="PSUM") as pgram,
                tc.tile_pool(name="ptr", bufs=2, space="PSUM") as ptr,
                tc.tile_pool(name="pmast", bufs=2, space="PSUM") as pmast,
            ):
              for b in range(BC):
                nT = npool.tile([DG, NUM_C], GRAM_DT, tag="nT")
                mast_ps = pmast.tile([1, DG], F32, tag="mast")
                for k in range(NCH):
                    cs = slice(k * 128, (k + 1) * 128)
                    mv_sb = chk.tile([128, DG], F32, tag="mv")
                    nc.gpsimd.dma_start(mv_sb[:], mv_d[b, cs, :])
                    ru_sb = chk.tile([128, DG], F32, tag="ru")
                    nc.gpsimd.dma_start(ru_sb[:], ru_d[b, cs, :])

                    # row norms -> normalized rows
                    sqn = tmp.tile([128, DG], F32, tag="sqn")
                    ssn = tmp.tile([128, 1], F32, tag="ssn")
                    nc.scalar.activation(sqn[:], mv_sb[:], ACTF.Square, accum_out=ssn[:])
                    snn = tmp.tile([128, 1], F32, tag="snn")
                    nc.scalar.activation(snn[:], ssn[:], ACTF.Sqrt, bias=1e-12)
                    rnn = tmp.tile([128, 1], F32, tag="rnn")
                    nc.vector.reciprocal(rnn[:], snn[:])
                    normed = tmp.tile([128, DG], F32, tag="normed")
                    nc.vector.tensor_scalar_mul(normed[:], mv_sb[:], rnn[:])
                    nT_ps = ptr.tile([DG, 128], F32, tag="tr")
                    nc.tensor.transpose(nT_ps[:], normed[:], eye[:])
                    nc.scalar.copy(nT[:, cs], nT_ps[:])

                    # forget gate + update
                    gj = tmp.tile([128, DG], F32, tag="gj")
                    gl = tmp.tile([128, 1], F32, tag="gl")
                    nc.vector.tensor_mul(gj[:], mv_sb[:], wrep[:])
                    nc.vector.tensor_reduce(gl[:], gj[:], axis=AX.X, op=ALU.add)
                    glog = tmp.tile([128, 1], F32, tag="glog")
                    nc.vector.tensor_scalar(
                        glog[:], tfT[:, k * BC + b:k * BC + b + 1], tf_coef, gl[:],
                        ALU.mult, ALU.add,
                    )
                    gamma = tmp.tile([128, 1], F32, tag="gamma")
                    nc.scalar.activation(gamma[:], glog[:], ACTF.Sigmoid, bias=b0)
                    dmr = tmp.tile([128, DG], F32, tag="dmr")
                    nc.vector.tensor_sub(dmr[:], mv_sb[:], ru_sb[:])
                    dmg = tmp.tile([128, DG], F32, tag="dmg")
                    nc.vector.tensor_scalar_mul(dmg[:], dmr[:], gamma[:])
                    upd = tmp.tile([128, DG], F32, tag="upd")
                    nc.vector.tensor_add(upd[:], dmg[:], ru_sb[:])
                    nc.gpsimd.dma_start(upd_d[b, cs, :], upd[:])

                    # mastery accumulation (attn-weighted sum of mv rows)
                    if _KV != "nomast":
                        nc.tensor.matmul(
                            mast_ps[:], wT[:, k * BC + b:k * BC + b + 1], mv_sb[:],
                            start=(k == 0), stop=(k == NCH - 1),
                        )
                if _KV != "nomast":
                    # psum -> sbuf row b (dma moves across partitions)
                    mtmp = tmp.tile([1, DG], F32, tag="mtmp")
                    nc.scalar.copy(mtmp[:], mast_ps[:])
                    nc.gpsimd.dma_start(mast_sb[b:b + 1, :], mtmp[:])
                elif b == 0:
                    nc.gpsimd.memset(mast_sb[:], 0.0)

                # gram: adj rows, 128 at a time
                for m in range(NCH if _KV != "nogram" else 0):
                    ms = slice(m * 128, (m + 1) * 128)
                    adj_row = adjp.tile([128, NUM_C], F32, tag="adjrow")
                    for h in range(2):
                        hs = slice(h * 512, (h + 1) * 512)
                        g_ps = pgram.tile([128, 512], F32, tag="gram")
                        nc.tensor.matmul(g_ps[:], nT[:, ms], nT[:, hs], start=True, stop=True)
                        nc.scalar.copy(adj_row[:, hs], g_ps[:])
                    mskg = adjp.tile([128, NUM_C], F32, tag="mskg")
                    nc.vector.tensor_single_scalar(mskg[:], adj_row[:], EDGE_TH, ALU.is_gt)
                    nc.vector.tensor_mul(adj_row[:], adj_row[:], mskg[:])
                    nc.vector.tensor_add(adj_row[:, ms], adj_row[:, ms], eye[:])
                    nc.sync.dma_start(adj_d[b, ms, :], adj_row[:])

            # ============== predictions ==============
            pj = cp.tile([BC, DG], F32)
            dotr = cp.tile([BC, 1], F32)
            nc.vector.tensor_mul(pj[:], pqn[:], mast_sb[:])
            nc.vector.tensor_reduce(dotr[:], pj[:], axis=AX.X, op=ALU.add)
            pred_sb = cp.tile([BC, 1], F32)
            nc.scalar.activation(pred_sb[:], dotr[:], ACTF.Sigmoid, scale=rs[:])
            nc.gpsimd.dma_start(pred_d[:], pred_sb[:])

    nc.compile()
    return nc


def kernel(q_idx, o_idx, u_idx, score, delta_t, response_update, memory_value, params):
    p = {k: np.asarray(v) for k, v in params.items()}
    q_idx = np.asarray(q_idx)
    delta_t = np.ascontiguousarray(np.asarray(delta_t, dtype=np.float32))
    response_update = np.ascontiguousarray(np.asarray(response_update, dtype=np.float32))
    memory_value = np.ascontiguousarray(np.asarray(memory_value, dtype=np.float32))

    # ---- host-side input prep (gathers / weight folding / layout) ----
    q = np.clip(q_idx, 0, NUM_Q - 1).astype(np.int64)
    qt = p["q_emb"][q].astype(np.float32)                      # [B, DQ]

    wv, bv = p["qst_wv"], p["qst_bv"]
    wo, bo = p["qst_wo"], p["qst_bo"]
    A = np.eye(DQ, dtype=np.float32) + wo @ wv                 # x = A @ qt + c
    c = wo @ bv + bo
    ln_g, ln_b = p["qst_ln_g"], p["qst_ln_b"]
    Wq = p["query_proj_w"] * ln_g[None, :]                     # fold LN affine
    bq = p["query_proj_b"] + p["query_proj_w"] @ ln_b

    mk = p["memory_key"]
    mkn = mk / np.clip(np.linalg.norm(mk, axis=-1, keepdims=True), 1e-12, None)

    wf = p["forget_w"][0]                                      # [DG+1]
    b0 = float(p["forget_b"][0])
    tf_coef = float(DECAY * wf[DG])
    wrep = np.ascontiguousarray(np.broadcast_to(wf[:DG], (128, DG)).astype(np.float32))
    eye128 = np.eye(128, dtype=np.float32)
    ones_row = np.ones((1, BC), dtype=np.float32)

    AT = np.ascontiguousarray(A.T).astype(np.float32)
    c_row = np.ascontiguousarray(c[None, :]).astype(np.float32)
    WqT = np.ascontiguousarray(Wq.T).astype(np.float32)
    bq_row = np.ascontiguousarray(bq[None, :]).astype(np.float32)
    mknT = np.ascontiguousarray(mkn.T).astype(np.float32)

    key = (b0, tf_coef)
    if key not in _CACHE:
        _CACHE[key] = _build(b0, tf_coef)
    nc = _CACHE[key]

    in_maps = []
    for i in range(NCORES):
        sl = slice(i * BC, (i + 1) * BC)
        in_maps.append({
            "qtT": np.ascontiguousarray(qt[sl].T),
            "AT": AT, "c_row": c_row, "WqT": WqT, "bq_row": bq_row,
            "mknT": mknT, "wrep": wrep, "eye": eye128, "ones_row": ones_row,
            "delta_t": np.ascontiguousarray(delta_t[sl]),
            "mv": np.ascontiguousarray(memory_value[sl]),
            "ru": np.ascontiguousarray(response_update[sl]),
        })

    trace = os.environ.get("KERNEL_TRACE", "0") == "1"
    res = run_bass_kernel_spmd(nc, in_maps, list(range(NCORES)), trace=trace)
    global LAST_EXEC_NS, LAST_RESULTS
    LAST_EXEC_NS = res.exec_time_ns
    LAST_RESULTS = res
    if trace and res.exec_time_ns is not None:
        print(f"HW exec time: {res.exec_time_ns} ns")

    pred = np.concatenate([res.results[i]["pred"][:, 0] for i in range(NCORES)], axis=0)
    adj = np.concatenate([res.results[i]["adj"] for i in range(NCORES)], axis=0)
    updated = np.concatenate([res.results[i]["upd"] for i in range(NCORES)], axis=0)
    return pred.astype(np.float32), adj.astype(np.float32), updated.astype(np.float32)
